# revision 21
# baseline (speedup 1.0000x reference)
"""BotRGCN Trainium2 kernel (8 NeuronCores, SPMD) — v2.

Sharding: nodes row-wise across 8 cores (12800 padded rows/core).
RGCN aggregation: dst-sorted dma_gather (4 source chunks — int16 index
reach) + segment-indicator matmuls on the Tensor engine (PSUM per
dst-group, SBUF f32 accumulator across the 4 chunk passes). No
dma_scatter_add — Q7 descriptor generation (the v1 bottleneck) is
halved. The edge layout (per-group slot capacities) is data-dependent,
so the program is built per input inside kernel(); host preprocessing
remains structural only (sharding, sorting, degree counts).

Self-contained: hardcodes N=100000, E=1600000, EMB=192, 2 relations.
"""

import os
import sys
from contextlib import ExitStack
from dataclasses import dataclass

import numpy as np
import ml_dtypes

for _p in ("/opt/trn_rl_repo",):
    if os.path.isdir(_p) and _p not in sys.path:
        sys.path.insert(0, _p)

import concourse.bass as bass
import concourse.mybir as mybir
from concourse import bacc, library_config, tile
from concourse.bass_utils import run_bass_kernel_spmd

F32 = mybir.dt.float32
BF16 = mybir.dt.bfloat16
F16 = mybir.dt.float16
I16 = mybir.dt.int16
AX = mybir.AluOpType
ACTF = mybir.ActivationFunctionType

LEAKY = 0.01
SEG_PAD = 600  # sentinel outside [0, 512)


@dataclass(frozen=True)
class Cfg:
    ncores: int = 8
    nsh: int = 12800           # padded nodes per core (mult of 2560)
    d_des: int = 768
    emb: int = 192
    third: int = 64
    src_chunk_cores: int = 2   # table rows per chunk <= 32767
    n_real: int = 100000
    sup_d: int = 2560          # dst nodes per super-block
    grp_d: int = 256           # dst nodes per psum group (512 slots)

    @property
    def n_per(self):
        return self.n_real // self.ncores

    @property
    def n_total(self):
        return self.ncores * self.nsh

    @property
    def nchunks(self):
        return self.ncores // self.src_chunk_cores

    @property
    def chunk_rows(self):
        return self.src_chunk_cores * self.nsh

    @property
    def nsup(self):
        return self.nsh // self.sup_d

    @property
    def ngrp(self):
        return self.sup_d // self.grp_d

    @property
    def node_chunks(self):
        return self.nsh // 512

    @property
    def kd(self):
        return self.d_des // 128


CFG = Cfg()


@dataclass(frozen=True)
class Layout:
    """Input-dependent edge layout, identical across cores."""
    wc: tuple            # wc[s][ch][g] -> window count (128 slots each)

    @property
    def total_windows(self):
        return sum(w for s in self.wc for c in s for w in c)

    @property
    def total_slots(self):
        return 128 * self.total_windows


# ----------------------------------------------------------------------------
# Device program
# ----------------------------------------------------------------------------

def build_program(C: Cfg, L: Layout):
    nc = bacc.Bacc(None, num_devices=C.ncores)

    P = 128
    EMB, TH = C.emb, C.third
    NSH = C.nsh
    NCH = C.node_chunks
    W = L.total_windows

    def param(name, shape, dtype=F32, out=False):
        return nc.declare_dram_parameter(name, list(shape), dtype, isOutput=out)

    desT = param("desT", (C.d_des, NSH))
    numT = param("numT", (4, NSH))
    catT = param("catT", (3, NSH))
    gidx = param("gidx", (P, L.total_slots // 16), I16)
    segp = param("segp", (P, W), F16)
    invp = param("invp", (P, W), F16)
    w_des = param("w_des", (C.d_des, TH))
    w_num = param("w_num", (4, TH))
    w_cat = param("w_cat", (3, TH))
    b0 = param("b0", (EMB,))
    w_in = param("w_in", (EMB, EMB))
    b_in = param("b_in", (EMB,))
    w_root = param("w_root", (EMB, EMB))
    w_rel0 = param("w_rel0", (EMB, EMB))
    w_rel1 = param("w_rel1", (EMB, EMB))
    b_rgcn = param("b_rgcn", (EMB,))
    w_o1 = param("w_o1", (EMB, EMB))
    b_o1 = param("b_o1", (EMB,))
    w_o2 = param("w_o2", (EMB, 2))
    b_o2r = param("b_o2r", (P, 2))
    out_p = param("out", (NSH, 2), out=True)

    agin1 = nc.dram_tensor("agin1", [NSH, EMB], F32)
    xg1 = nc.dram_tensor("xg1", [C.n_total, EMB], F32, addr_space="Shared")
    agin2 = nc.dram_tensor("agin2", [NSH, EMB], F32)
    xg2 = nc.dram_tensor("xg2", [C.n_total, EMB], F32, addr_space="Shared")

    replica = [list(range(C.ncores))]

    with tile.TileContext(nc) as tc, ExitStack() as top:
        nc.gpsimd.load_library(library_config.mlp)
        const = top.enter_context(tc.tile_pool(name="const", bufs=1))

        def cload(src_ap, shape, name, dtype=F32):
            t = const.tile(list(shape), dtype, tag=name)
            nc.sync.dma_start(out=t[:], in_=src_ap)
            return t

        ident_dram = nc.inline_tensor(np.eye(P, dtype=np.float32),
                                      name="identity128")
        ident = cload(ident_dram[:, :], (P, P), "ident")

        def load_ab(w, name):
            a = cload(w[0:P, :], (P, int(w.shape[1])), name + "A")
            b = cload(w[P:EMB, :], (EMB - P, int(w.shape[1])), name + "B")
            return a, b

        def load_lohi(w, name):
            lo = cload(w[0:96, :], (96, int(w.shape[1])), name + "lo")
            hi = cload(w[96:EMB, :], (96, int(w.shape[1])), name + "hi")
            return lo, hi

        wdes = cload(w_des[:, :].rearrange("(k p) m -> p k m", p=P),
                     (P, C.kd, TH), "wdes")
        wnum = cload(w_num[:, :], (4, TH), "wnum")
        wcat = cload(w_cat[:, :], (3, TH), "wcat")
        winA, winB = load_ab(w_in, "win")
        wrootA, wrootB = load_ab(w_root, "wroot")
        wrel = [load_lohi(w_rel0, "wrel0"), load_lohi(w_rel1, "wrel1")]
        wo1A, wo1B = load_ab(w_o1, "wo1")
        wo2A = cload(w_o2[0:P, :], (P, 2), "wo2A")
        wo2B = cload(w_o2[P:EMB, :], (EMB - P, 2), "wo2B")

        def load_colvec(v, name):
            a = const.tile([P, 1], F32, tag=name + "A")
            b = const.tile([EMB - P, 1], F32, tag=name + "B")
            nc.sync.dma_start(out=a[:], in_=v[0:P].unsqueeze(1))
            nc.sync.dma_start(out=b[:], in_=v[P:EMB].unsqueeze(1))
            return a, b

        b0A, b0B = load_colvec(b0, "b0")
        binA, binB = load_colvec(b_in, "bin")
        brgA, brgB = load_colvec(b_rgcn, "brg")
        bo1A, bo1B = load_colvec(b_o1, "bo1")
        bo2 = cload(b_o2r[:, :], (P, 2), "bo2")

        seg_sb = const.tile([P, W], F16, tag="seg")
        nc.sync.dma_start(out=seg_sb[:], in_=segp[:, :])
        inv_sb = const.tile([P, W], F16, tag="inv")
        nc.sync.dma_start(out=inv_sb[:], in_=invp[:, :])

        iota_dram = nc.inline_tensor(
            np.tile(np.arange(512, dtype=np.float16)[None, None, :],
                    (P, 8, 1)),
            name="iota512")
        iota = cload(iota_dram[:, :, :], (P, 8, 512), "iota", dtype=F16)

        outsb = const.tile([P, NSH // P, 2], F32, tag="outsb")

        def leaky_(ap):
            nc.vector.scalar_tensor_tensor(ap, ap, LEAKY, ap,
                                           op0=AX.mult, op1=AX.max)

        def act_bias(out_ap, in_ap, bias_ap):
            nc.scalar.activation(out_ap, in_ap, ACTF.Identity,
                                 bias=bias_ap, scale=1.0)

        def mm(out_ap, l_ap, r_ap, first=False, last=False):
            nc.tensor.matmul(out_ap, l_ap, r_ap, start=first, stop=last)

        def emit_node_major(pool, psum, xa_c, xb_c, dst, c, tagA="ptA"):
            ptA = psum.tile([P, 512], F32, tag=tagA, name="ptA")
            ptB = psum.tile([P, 256], F32, tag="ptB")
            for t in range(4):
                nc.tensor.matmul(ptA[:, bass.ts(t, P)],
                                 xa_c[:, bass.ts(t, P)], ident[:],
                                 is_transpose=True)
                nc.tensor.matmul(ptB[:, bass.ts(t, TH)],
                                 xb_c[:, bass.ts(t, P)], ident[0:TH, 0:TH],
                                 is_transpose=True)
            agsb = pool.tile([P, 4, EMB], F32, tag="agsb")
            nc.vector.tensor_copy(
                agsb[:, :, 0:P], ptA[:].rearrange("p (t f) -> p t f", f=P))
            nc.vector.tensor_copy(
                agsb[:, :, P:EMB], ptB[:].rearrange("p (t f) -> p t f", f=TH))
            nc.sync.dma_start(
                out=dst[c * 512:(c + 1) * 512, :].rearrange(
                    "(t p) f -> p t f", p=P),
                in_=agsb[:])

        def load_xT(pool, psum, src, c):
            nm = pool.tile([P, 4, EMB], F32, tag="nm")
            nc.sync.dma_start(
                out=nm[:],
                in_=src[c * 512:(c + 1) * 512, :].rearrange(
                    "(t p) f -> p t f", p=P))
            pA = psum.tile([P, 512], F32, tag="oA")
            pB = psum.tile([TH, 512], F32, tag="oB")
            for t in range(4):
                nc.tensor.matmul(pA[:, bass.ts(t, P)], nm[:, t, 0:P],
                                 ident[:], is_transpose=True)
                nc.tensor.matmul(pB[:, bass.ts(t, P)], nm[:, t, P:EMB],
                                 ident[:], is_transpose=True)
            xa = pool.tile([P, 512], F32, tag="xTa")
            xb = pool.tile([TH, 512], F32, tag="xTb")
            nc.vector.tensor_copy(xa[:], pA[:])
            nc.vector.tensor_copy(xb[:], pB[:])
            return xa, xb

        # ------------------------------------------------------------
        # Phase A: x1 = leaky(leaky(feats @ W_*) @ W_in + b_in)
        # ------------------------------------------------------------
        with tc.tile_pool(name="pa", bufs=2) as pa, \
             tc.tile_pool(name="paps", bufs=1, space="PSUM") as paps, \
             tc.tile_pool(name="panp", bufs=1) as panp:
            np_sb = panp.tile([4, NSH], F32, tag="np")
            cp_sb = panp.tile([3, NSH], F32, tag="cp")
            nc.sync.dma_start(out=np_sb[:], in_=numT[:, :])
            nc.sync.dma_start(out=cp_sb[:], in_=catT[:, :])
            desT_r = desT[:, :].rearrange("(k p) n -> p k n", p=P)

            for c in range(NCH):
                sl = bass.ts(c, 512)
                des_c = pa.tile([P, C.kd, 512], F32, tag="des")
                nc.sync.dma_start(out=des_c[:], in_=desT_r[:, :, sl])
                psA = paps.tile([P, 512], F32, tag="psA")
                psB = paps.tile([TH, 512], F32, tag="psB")
                for k in range(C.kd):
                    nc.tensor.matmul(psA[0:TH, :], wdes[:, k, :],
                                     des_c[:, k, :],
                                     start=(k == 0), stop=(k == C.kd - 1))
                nc.tensor.matmul(psA[TH:P, :], wnum[:], np_sb[:, sl],
                                 start=True, stop=True)
                nc.tensor.matmul(psB[:], wcat[:], cp_sb[:, sl],
                                 start=True, stop=True)
                x0A = pa.tile([P, 512], F32, tag="x0A")
                x0B = pa.tile([TH, 512], F32, tag="x0B")
                act_bias(x0A[:], psA[:], b0A[:])
                act_bias(x0B[:], psB[:], b0B[:])
                leaky_(x0A[:])
                leaky_(x0B[:])
                ps1A = paps.tile([P, 512], F32, tag="ps1A")
                ps1B = paps.tile([TH, 512], F32, tag="ps1B")
                mm(ps1A[:], winA[:, 0:P], x0A[:], first=True)
                mm(ps1A[:], winB[:, 0:P], x0B[:], last=True)
                mm(ps1B[:], winA[:, P:EMB], x0A[:], first=True)
                mm(ps1B[:], winB[:, P:EMB], x0B[:], last=True)
                x1A = pa.tile([P, 512], F32, tag="x1A")
                x1B = pa.tile([TH, 512], F32, tag="x1B")
                act_bias(x1A[:], ps1A[:], binA[:])
                act_bias(x1B[:], ps1B[:], binB[:])
                leaky_(x1A[:])
                leaky_(x1B[:])
                emit_node_major(pa, paps, x1A[:], x1B[:], agin1, c)

        # ------------------------------------------------------------
        # RGCN layer: gather + indicator-matmul aggregation + consume
        # ------------------------------------------------------------
        def layer(lid, agin, xg, consume):
            wc = L.wc
            with tc.tile_pool(name=f"gb{lid}", bufs=4) as gb, \
                 tc.tile_pool(name=f"gi{lid}", bufs=4) as gip, \
                 tc.tile_pool(name=f"ind{lid}", bufs=3) as indp, \
                 tc.tile_pool(name=f"agg{lid}", bufs=1) as aggp, \
                 tc.tile_pool(name=f"agps{lid}", bufs=2, space="PSUM") as agps, \
                 tc.tile_pool(name=f"mmps{lid}", bufs=1, space="PSUM") as mmps, \
                 tc.tile_pool(name=f"xt{lid}", bufs=2) as xt, \
                 tc.tile_pool(name=f"cons{lid}", bufs=2) as cpool:
                woff = 0      # global window index
                soff = 0      # global slot offset (for gidx columns)
                for s in range(C.nsup):
                    accA = aggp.tile([96, C.ngrp * 512], F32, tag="accA")
                    accB = aggp.tile([96, C.ngrp * 512], F32, tag="accB")
                    for ch in range(C.nchunks):
                        xg_sl = xg[ch * C.chunk_rows:(ch + 1) * C.chunk_rows, :]
                        wlist = []   # (group, first, last) per window
                        for g in range(C.ngrp):
                            n = wc[s][ch][g]
                            for i in range(n):
                                wlist.append((g, i == 0, i == n - 1))
                        # batches of up to 8 windows
                        b = 0
                        psg = {}
                        while b < len(wlist):
                            k = min(8, len(wlist) - b)
                            nb = k * 128
                            gi = gip.tile([P, nb // 16], I16, tag="gi")
                            nc.sync.dma_start(
                                out=gi[:],
                                in_=gidx[:, soff // 16:(soff + nb) // 16])
                            bt = gb.tile([P, k, EMB], F32, tag="bt")
                            nc.gpsimd.dma_gather(bt[:], xg_sl, gi[:],
                                                 nb, nb, EMB)
                            btf = gb.tile([P, k, EMB], F16, tag="btf")
                            nc.scalar.activation(btf[:], bt[:], ACTF.Identity)
                            wsl = slice(woff + b, woff + b + k)
                            nc.vector.tensor_tensor(
                                btf[:], btf[:],
                                inv_sb[:, wsl].unsqueeze(-1).broadcast_to(
                                    [P, k, EMB]),
                                op=AX.mult)
                            ind = indp.tile([P, k, 512], F16, tag="ind")
                            nc.vector.tensor_tensor(
                                ind[:],
                                seg_sb[:, wsl].unsqueeze(-1).broadcast_to(
                                    [P, k, 512]),
                                iota[:, 0:k, :],
                                op=AX.is_equal)
                            for j in range(k):
                                g, first, last = wlist[b + j]
                                if first:
                                    psg[g] = (agps.tile([96, 512], F32,
                                                        tag="psA", name="psA"),
                                              agps.tile([96, 512], F32,
                                                        tag="psB", name="psB"))
                                pA, pB = psg[g]
                                nc.tensor.matmul(pA[:], btf[:, j, 0:96],
                                                 ind[:, j, :],
                                                 start=first, stop=last)
                                nc.tensor.matmul(pB[:], btf[:, j, 96:EMB],
                                                 ind[:, j, :],
                                                 start=first, stop=last)
                                if last:
                                    dst = slice(g * 512, (g + 1) * 512)
                                    if ch == 0:
                                        nc.vector.tensor_copy(
                                            accA[:, dst], pA[:])
                                        nc.vector.tensor_copy(
                                            accB[:, dst], pB[:])
                                    else:
                                        nc.vector.tensor_tensor(
                                            accA[:, dst], accA[:, dst],
                                            pA[:], op=AX.add)
                                        nc.vector.tensor_tensor(
                                            accB[:, dst], accB[:, dst],
                                            pB[:], op=AX.add)
                                    del psg[g]
                            soff += nb
                            b += k
                        woff += len(wlist)
                    # consume super s: 512-node chunks
                    for i in range(C.sup_d // 512):
                        c = s * (C.sup_d // 512) + i
                        xa, xb = load_xT(xt, mmps, agin, c)
                        oA = mmps.tile([P, 512], F32, tag="oA")
                        oB = mmps.tile([TH, 512], F32, tag="oB")
                        mm(oA[:], wrootA[:, 0:P], xa[:], first=True)
                        mm(oA[:], wrootB[:, 0:P], xb[:])
                        mm(oB[:], wrootA[:, P:EMB], xa[:], first=True)
                        mm(oB[:], wrootB[:, P:EMB], xb[:])
                        for r in range(2):
                            lo, hi = wrel[r]
                            for piece, acc in ((lo, accA), (hi, accB)):
                                last = (r == 1 and piece is hi)
                                for h in range(2):
                                    g2 = 2 * i + h
                                    csl = slice(g2 * 512 + r * 256,
                                                g2 * 512 + r * 256 + 256)
                                    mm(oA[:, bass.ts(h, 256)], piece[:, 0:P],
                                       acc[:, csl], last=last)
                                for h in range(2):
                                    g2 = 2 * i + h
                                    csl = slice(g2 * 512 + r * 256,
                                                g2 * 512 + r * 256 + 256)
                                    mm(oB[:, bass.ts(h, 256)], piece[:, P:EMB],
                                       acc[:, csl], last=last)
                        consume(c, oA, oB, cpool, mmps)

        # ---- layer 1 ----
        nc.gpsimd.collective_compute(
            "AllGather", AX.bypass, replica_groups=replica,
            ins=[agin1[:, :].opt()], outs=[xg1[:, :].opt()])

        def consume1(c, oA, oB, cpool, cpps):
            x2a = cpool.tile([P, 512], F32, tag="x2a")
            x2b = cpool.tile([TH, 512], F32, tag="x2b")
            act_bias(x2a[:], oA[:], brgA[:])
            act_bias(x2b[:], oB[:], brgB[:])
            emit_node_major(cpool, cpps, x2a[:], x2b[:], agin2, c, tagA="oA")

        layer(1, agin1, xg1, consume1)

        # ---- layer 2 (+ fused output head) ----
        nc.gpsimd.collective_compute(
            "AllGather", AX.bypass, replica_groups=replica,
            ins=[agin2[:, :].opt()], outs=[xg2[:, :].opt()])

        def consume2(c, oA, oB, cpool, cpps):
            x3A = cpool.tile([P, 512], F32, tag="x3A")
            x3B = cpool.tile([TH, 512], F32, tag="x3B")
            act_bias(x3A[:], oA[:], brgA[:])
            act_bias(x3B[:], oB[:], brgB[:])
            p1A = cpps.tile([P, 512], F32, tag="oA")
            p1B = cpps.tile([TH, 512], F32, tag="oB")
            mm(p1A[:], wo1A[:, 0:P], x3A[:], first=True)
            mm(p1A[:], wo1B[:, 0:P], x3B[:], last=True)
            mm(p1B[:], wo1A[:, P:EMB], x3A[:], first=True)
            mm(p1B[:], wo1B[:, P:EMB], x3B[:], last=True)
            o1A = cpool.tile([P, 512], F32, tag="o1A")
            o1B = cpool.tile([TH, 512], F32, tag="o1B")
            act_bias(o1A[:], p1A[:], bo1A[:])
            act_bias(o1B[:], p1B[:], bo1B[:])
            leaky_(o1A[:])
            leaky_(o1B[:])
            for t in range(4):
                pso = cpps.tile([P, 2], F32, tag="pso")
                mm(pso[:], o1A[:, bass.ts(t, P)], wo2A[:], first=True)
                mm(pso[:], o1B[:, bass.ts(t, P)], wo2B[:], last=True)
                nc.vector.tensor_tensor(outsb[:, c * 4 + t, :], pso[:],
                                        bo2[:], op=AX.add)

        layer(2, agin2, xg2, consume2)

        nc.sync.dma_start(
            out=out_p[:, :].rearrange("(t p) c -> p t c", p=P),
            in_=outsb[:])

    return nc


# ----------------------------------------------------------------------------
# Host-side structural preprocessing
# ----------------------------------------------------------------------------

def _wrap16(a):
    w = a.reshape(-1, 16).T.astype(np.int16)
    return np.ascontiguousarray(np.tile(w, (8, 1)))


def prep_inputs(C: Cfg, inputs):
    des = np.asarray(inputs["des"], np.float32)
    num_prop = np.asarray(inputs["num_prop"], np.float32)
    cat_prop = np.asarray(inputs["cat_prop"], np.float32)
    ei = np.asarray(inputs["edge_index"]).astype(np.int64)
    et = np.asarray(inputs["edge_type"]).astype(np.int64)
    src, dst = ei[0], ei[1]

    w_rel = np.asarray(inputs["W_rel"], np.float32)
    shared = {
        "w_des": np.asarray(inputs["W_des"], np.float32),
        "w_num": np.asarray(inputs["W_num"], np.float32),
        "w_cat": np.asarray(inputs["W_cat"], np.float32),
        "b0": np.concatenate([np.asarray(inputs["b_des"], np.float32),
                              np.asarray(inputs["b_num"], np.float32),
                              np.asarray(inputs["b_cat"], np.float32)]),
        "w_in": np.asarray(inputs["W_in"], np.float32),
        "b_in": np.asarray(inputs["b_in"], np.float32),
        "w_root": np.asarray(inputs["W_root"], np.float32),
        "w_rel0": np.ascontiguousarray(w_rel[0]),
        "w_rel1": np.ascontiguousarray(w_rel[1]),
        "b_rgcn": np.asarray(inputs["b_rgcn"], np.float32),
        "w_o1": np.asarray(inputs["W_o1"], np.float32),
        "b_o1": np.asarray(inputs["b_o1"], np.float32),
        "w_o2": np.asarray(inputs["W_o2"], np.float32),
        "b_o2r": np.tile(np.asarray(inputs["b_o2"], np.float32)[None, :],
                         (128, 1)),
    }

    n_per = C.n_per
    src_pad = (src // n_per) * C.nsh + (src % n_per)
    chunk_all = (src_pad // C.chunk_rows).astype(np.int64)
    src_loc_all = (src_pad % C.chunk_rows).astype(np.int32)
    dst_core = dst // n_per
    dst_loc_all = (dst % n_per).astype(np.int64)

    NS, NG, NC_ = C.nsup, C.ngrp, C.nchunks
    per_core = []
    cnts = np.zeros((C.ncores, NS, NC_, NG), np.int64)
    for i in range(C.ncores):
        m = dst_core == i
        dl = dst_loc_all[m]
        ch = chunk_all[m]
        sl = src_loc_all[m]
        r = et[m]
        sup = dl // C.sup_d
        grp = (dl % C.sup_d) // C.grp_d
        slot = (r * C.grp_d + (dl % C.grp_d)).astype(np.int64)
        # per (dst, rel) in-degree -> inv
        cnt = np.bincount(dl * 2 + r, minlength=2 * C.nsh)
        inv = (1.0 / np.maximum(cnt, 1)).astype(np.float32)
        inv_e = inv[dl * 2 + r]
        order = np.lexsort((slot, grp, ch, sup))
        key = ((sup * NC_ + ch) * NG + grp)
        np.add.at(cnts[i].reshape(-1), key, 1)
        per_core.append((sl[order], inv_e[order], slot[order],
                         key[order]))

    caps = 128 * np.maximum(
        np.ceil(cnts.max(axis=0) / 128).astype(np.int64), 1)  # [NS,NC_,NG]
    wc = tuple(tuple(tuple(int(x) // 128 for x in c) for c in s)
               for s in caps)
    L = Layout(wc=wc)
    total = int(caps.sum())

    offs = np.zeros(NS * NC_ * NG + 1, np.int64)
    np.cumsum(caps.reshape(-1), out=offs[1:])

    in_maps = []
    for i in range(C.ncores):
        rs, re = i * n_per, (i + 1) * n_per
        desT = np.zeros((C.d_des, C.nsh), np.float32)
        desT[:, :n_per] = des[rs:re].T
        numT = np.zeros((4, C.nsh), np.float32)
        numT[:, :n_per] = num_prop[rs:re].T
        catT = np.zeros((3, C.nsh), np.float32)
        catT[:, :n_per] = cat_prop[rs:re].T

        sl, inv_e, slot, key = per_core[i]
        ne = len(sl)
        # position within run
        kcnt = np.bincount(key, minlength=NS * NC_ * NG)
        kstart = np.zeros(NS * NC_ * NG + 1, np.int64)
        np.cumsum(kcnt, out=kstart[1:])
        pos = np.arange(ne) - kstart[key]
        gpos = offs[key] + pos

        gflat = np.zeros(total, np.int32)
        segf = np.full(total, SEG_PAD, np.float32)
        invf = np.zeros(total, np.float32)
        gflat[gpos] = sl
        segf[gpos] = slot
        invf[gpos] = inv_e

        gidx = _wrap16(gflat)
        segp = np.ascontiguousarray(
            segf.reshape(-1, 128).T.astype(np.float16))  # [128, W]
        invp = np.ascontiguousarray(
            invf.reshape(-1, 128).T.astype(np.float16))

        in_maps.append(dict(shared, desT=desT, numT=numT, catT=catT,
                            gidx=gidx, segp=segp, invp=invp))
    return L, in_maps


_PROG_CACHE = {}


def _get_program(C: Cfg, L: Layout):
    key = (C, L)
    if key not in _PROG_CACHE:
        nc = build_program(C, L)
        nc.finalize()
        _PROG_CACHE[key] = nc
    return _PROG_CACHE[key]


def _install_ntff_shim():
    import types
    try:
        import antenv.axon_hooks  # noqa: F401
        return
    except ImportError:
        pass
    try:
        from trn_agent_boot.trn_boot import _ntff_profile_via_ctypes
        hook = _ntff_profile_via_ctypes("/opt/axon/libaxon_pjrt.so")
    except Exception:
        hook = None
    mod = types.ModuleType("antenv.axon_hooks")
    mod.get_axon_ntff_profile_hook = lambda: hook
    mod.set_axon_ntff_profile_hook = lambda h: None
    sys.modules["antenv.axon_hooks"] = mod


def run_on_hw(inputs, trace=False, trace_kwargs=None):
    C = CFG
    L, in_maps = prep_inputs(C, inputs)
    nc = _get_program(C, L)
    if trace:
        _install_ntff_shim()
    res = run_bass_kernel_spmd(nc, in_maps, list(range(C.ncores)),
                               trace=trace, **(trace_kwargs or {}))
    outs = [res.results[i]["out"][:C.n_per] for i in range(C.ncores)]
    full = np.concatenate(outs, axis=0)
    return full, res


def kernel(**inputs):
    full, _ = run_on_hw(inputs, trace=False)
    return full


# revision 22
# speedup vs baseline: 1.0140x; 1.0140x over previous
"""BotRGCN Trainium2 kernel (8 NeuronCores, SPMD) — v2.

Sharding: nodes row-wise across 8 cores (12800 padded rows/core).
RGCN aggregation: dst-sorted dma_gather (4 source chunks — int16 index
reach) + segment-indicator matmuls on the Tensor engine (PSUM per
dst-group, SBUF f32 accumulator across the 4 chunk passes). No
dma_scatter_add — Q7 descriptor generation (the v1 bottleneck) is
halved. The edge layout (per-group slot capacities) is data-dependent,
so the program is built per input inside kernel(); host preprocessing
remains structural only (sharding, sorting, degree counts).

Self-contained: hardcodes N=100000, E=1600000, EMB=192, 2 relations.
"""

import os
import sys
from contextlib import ExitStack
from dataclasses import dataclass

import numpy as np
import ml_dtypes

for _p in ("/opt/trn_rl_repo",):
    if os.path.isdir(_p) and _p not in sys.path:
        sys.path.insert(0, _p)

import concourse.bass as bass
import concourse.mybir as mybir
from concourse import bacc, library_config, tile
from concourse.bass_utils import run_bass_kernel_spmd

F32 = mybir.dt.float32
BF16 = mybir.dt.bfloat16
F16 = mybir.dt.float16
I16 = mybir.dt.int16
AX = mybir.AluOpType
ACTF = mybir.ActivationFunctionType

LEAKY = 0.01
SEG_PAD = 600  # sentinel outside [0, 512)


@dataclass(frozen=True)
class Cfg:
    ncores: int = 8
    nsh: int = 12800           # padded nodes per core (mult of 2560)
    d_des: int = 768
    emb: int = 192
    third: int = 64
    src_chunk_cores: int = 2   # table rows per chunk <= 32767
    n_real: int = 100000
    sup_d: int = 2560          # dst nodes per super-block
    grp_d: int = 256           # dst nodes per psum group (512 slots)

    @property
    def n_per(self):
        return self.n_real // self.ncores

    @property
    def n_total(self):
        return self.ncores * self.nsh

    @property
    def nchunks(self):
        return self.ncores // self.src_chunk_cores

    @property
    def chunk_rows(self):
        return self.src_chunk_cores * self.nsh

    @property
    def nsup(self):
        return self.nsh // self.sup_d

    @property
    def ngrp(self):
        return self.sup_d // self.grp_d

    @property
    def node_chunks(self):
        return self.nsh // 512

    @property
    def kd(self):
        return self.d_des // 128


CFG = Cfg()


@dataclass(frozen=True)
class Layout:
    """Input-dependent edge layout, identical across cores."""
    wc: tuple            # wc[s][ch][g] -> window count (128 slots each)

    @property
    def total_windows(self):
        return sum(w for s in self.wc for c in s for w in c)

    @property
    def total_slots(self):
        return 128 * self.total_windows


# ----------------------------------------------------------------------------
# Device program
# ----------------------------------------------------------------------------

def build_program(C: Cfg, L: Layout):
    nc = bacc.Bacc(None, num_devices=C.ncores)

    P = 128
    EMB, TH = C.emb, C.third
    NSH = C.nsh
    NCH = C.node_chunks
    W = L.total_windows

    def param(name, shape, dtype=F32, out=False):
        return nc.declare_dram_parameter(name, list(shape), dtype, isOutput=out)

    desT = param("desT", (C.d_des, NSH))
    numT = param("numT", (4, NSH))
    catT = param("catT", (3, NSH))
    gidx = param("gidx", (P, L.total_slots // 16), I16)
    segp = param("segp", (P, W), F16)
    invp = param("invp", (P, W), F16)
    w_des = param("w_des", (C.d_des, TH))
    w_num = param("w_num", (4, TH))
    w_cat = param("w_cat", (3, TH))
    b0 = param("b0", (EMB,))
    w_in = param("w_in", (EMB, EMB))
    b_in = param("b_in", (EMB,))
    w_root = param("w_root", (EMB, EMB))
    w_rel0 = param("w_rel0", (EMB, EMB))
    w_rel1 = param("w_rel1", (EMB, EMB))
    b_rgcn = param("b_rgcn", (EMB,))
    w_o1 = param("w_o1", (EMB, EMB))
    b_o1 = param("b_o1", (EMB,))
    w_o2 = param("w_o2", (EMB, 2))
    b_o2r = param("b_o2r", (P, 2))
    out_p = param("out", (NSH, 2), out=True)

    agin1 = nc.dram_tensor("agin1", [NSH, EMB], F32)
    xg1 = nc.dram_tensor("xg1", [C.n_total, EMB], F32, addr_space="Shared")
    agin2 = nc.dram_tensor("agin2", [NSH, EMB], F32)
    xg2 = nc.dram_tensor("xg2", [C.n_total, EMB], F32, addr_space="Shared")

    replica = [list(range(C.ncores))]

    with tile.TileContext(nc) as tc, ExitStack() as top:
        nc.gpsimd.load_library(library_config.mlp)
        const = top.enter_context(tc.tile_pool(name="const", bufs=1))

        def cload(src_ap, shape, name, dtype=F32):
            t = const.tile(list(shape), dtype, tag=name)
            nc.sync.dma_start(out=t[:], in_=src_ap)
            return t

        ident_dram = nc.inline_tensor(np.eye(P, dtype=np.float32),
                                      name="identity128")
        ident = cload(ident_dram[:, :], (P, P), "ident")

        def load_ab(w, name):
            a = cload(w[0:P, :], (P, int(w.shape[1])), name + "A")
            b = cload(w[P:EMB, :], (EMB - P, int(w.shape[1])), name + "B")
            return a, b

        def load_lohi(w, name):
            lo = cload(w[0:96, :], (96, int(w.shape[1])), name + "lo")
            hi = cload(w[96:EMB, :], (96, int(w.shape[1])), name + "hi")
            return lo, hi

        wdes = cload(w_des[:, :].rearrange("(k p) m -> p k m", p=P),
                     (P, C.kd, TH), "wdes")
        wnum = cload(w_num[:, :], (4, TH), "wnum")
        wcat = cload(w_cat[:, :], (3, TH), "wcat")
        winA, winB = load_ab(w_in, "win")
        wrootA, wrootB = load_ab(w_root, "wroot")
        wrel = [load_lohi(w_rel0, "wrel0"), load_lohi(w_rel1, "wrel1")]
        wo1A, wo1B = load_ab(w_o1, "wo1")
        wo2A = cload(w_o2[0:P, :], (P, 2), "wo2A")
        wo2B = cload(w_o2[P:EMB, :], (EMB - P, 2), "wo2B")

        def load_colvec(v, name):
            a = const.tile([P, 1], F32, tag=name + "A")
            b = const.tile([EMB - P, 1], F32, tag=name + "B")
            nc.sync.dma_start(out=a[:], in_=v[0:P].unsqueeze(1))
            nc.sync.dma_start(out=b[:], in_=v[P:EMB].unsqueeze(1))
            return a, b

        b0A, b0B = load_colvec(b0, "b0")
        binA, binB = load_colvec(b_in, "bin")
        brgA, brgB = load_colvec(b_rgcn, "brg")
        bo1A, bo1B = load_colvec(b_o1, "bo1")
        bo2 = cload(b_o2r[:, :], (P, 2), "bo2")

        seg_sb = const.tile([P, W], F16, tag="seg")
        nc.sync.dma_start(out=seg_sb[:], in_=segp[:, :])
        inv_sb = const.tile([P, W], F16, tag="inv")
        nc.sync.dma_start(out=inv_sb[:], in_=invp[:, :])

        iota_dram = nc.inline_tensor(
            np.tile(np.arange(512, dtype=np.float16)[None, None, :],
                    (P, 8, 1)),
            name="iota512")
        iota = cload(iota_dram[:, :, :], (P, 8, 512), "iota", dtype=F16)

        outsb = const.tile([P, NSH // P, 2], F32, tag="outsb")

        def leaky_(ap):
            nc.vector.scalar_tensor_tensor(ap, ap, LEAKY, ap,
                                           op0=AX.mult, op1=AX.max)

        def act_bias(out_ap, in_ap, bias_ap):
            nc.scalar.activation(out_ap, in_ap, ACTF.Identity,
                                 bias=bias_ap, scale=1.0)

        def mm(out_ap, l_ap, r_ap, first=False, last=False):
            nc.tensor.matmul(out_ap, l_ap, r_ap, start=first, stop=last)

        def emit_node_major(pool, psum, xa_c, xb_c, dst, c, tagA="ptA"):
            ptA = psum.tile([P, 512], F32, tag=tagA, name="ptA")
            ptB = psum.tile([P, 256], F32, tag="ptB")
            for t in range(4):
                nc.tensor.matmul(ptA[:, bass.ts(t, P)],
                                 xa_c[:, bass.ts(t, P)], ident[:],
                                 is_transpose=True)
                nc.tensor.matmul(ptB[:, bass.ts(t, TH)],
                                 xb_c[:, bass.ts(t, P)], ident[0:TH, 0:TH],
                                 is_transpose=True)
            agsb = pool.tile([P, 4, EMB], F32, tag="agsb")
            nc.vector.tensor_copy(
                agsb[:, :, 0:P], ptA[:].rearrange("p (t f) -> p t f", f=P))
            nc.vector.tensor_copy(
                agsb[:, :, P:EMB], ptB[:].rearrange("p (t f) -> p t f", f=TH))
            nc.sync.dma_start(
                out=dst[c * 512:(c + 1) * 512, :].rearrange(
                    "(t p) f -> p t f", p=P),
                in_=agsb[:])

        def load_xT(pool, psum, src, c):
            nm = pool.tile([P, 4, EMB], F32, tag="nm")
            nc.sync.dma_start(
                out=nm[:],
                in_=src[c * 512:(c + 1) * 512, :].rearrange(
                    "(t p) f -> p t f", p=P))
            pA = psum.tile([P, 512], F32, tag="oA")
            pB = psum.tile([TH, 512], F32, tag="oB")
            for t in range(4):
                nc.tensor.matmul(pA[:, bass.ts(t, P)], nm[:, t, 0:P],
                                 ident[:], is_transpose=True)
                nc.tensor.matmul(pB[:, bass.ts(t, P)], nm[:, t, P:EMB],
                                 ident[:], is_transpose=True)
            xa = pool.tile([P, 512], F32, tag="xTa")
            xb = pool.tile([TH, 512], F32, tag="xTb")
            nc.vector.tensor_copy(xa[:], pA[:])
            nc.vector.tensor_copy(xb[:], pB[:])
            return xa, xb

        # ------------------------------------------------------------
        # Phase A: x1 = leaky(leaky(feats @ W_*) @ W_in + b_in)
        # ------------------------------------------------------------
        with tc.tile_pool(name="pa", bufs=2) as pa, \
             tc.tile_pool(name="paps", bufs=1, space="PSUM") as paps, \
             tc.tile_pool(name="panp", bufs=1) as panp:
            np_sb = panp.tile([4, NSH], F32, tag="np")
            cp_sb = panp.tile([3, NSH], F32, tag="cp")
            nc.sync.dma_start(out=np_sb[:], in_=numT[:, :])
            nc.sync.dma_start(out=cp_sb[:], in_=catT[:, :])
            desT_r = desT[:, :].rearrange("(k p) n -> p k n", p=P)

            for c in range(NCH):
                sl = bass.ts(c, 512)
                des_c = pa.tile([P, C.kd, 512], F32, tag="des")
                nc.sync.dma_start(out=des_c[:], in_=desT_r[:, :, sl])
                psA = paps.tile([P, 512], F32, tag="psA")
                psB = paps.tile([TH, 512], F32, tag="psB")
                for k in range(C.kd):
                    nc.tensor.matmul(psA[0:TH, :], wdes[:, k, :],
                                     des_c[:, k, :],
                                     start=(k == 0), stop=(k == C.kd - 1))
                nc.tensor.matmul(psA[TH:P, :], wnum[:], np_sb[:, sl],
                                 start=True, stop=True)
                nc.tensor.matmul(psB[:], wcat[:], cp_sb[:, sl],
                                 start=True, stop=True)
                x0A = pa.tile([P, 512], F32, tag="x0A")
                x0B = pa.tile([TH, 512], F32, tag="x0B")
                act_bias(x0A[:], psA[:], b0A[:])
                act_bias(x0B[:], psB[:], b0B[:])
                leaky_(x0A[:])
                leaky_(x0B[:])
                ps1A = paps.tile([P, 512], F32, tag="ps1A")
                ps1B = paps.tile([TH, 512], F32, tag="ps1B")
                mm(ps1A[:], winA[:, 0:P], x0A[:], first=True)
                mm(ps1A[:], winB[:, 0:P], x0B[:], last=True)
                mm(ps1B[:], winA[:, P:EMB], x0A[:], first=True)
                mm(ps1B[:], winB[:, P:EMB], x0B[:], last=True)
                x1A = pa.tile([P, 512], F32, tag="x1A")
                x1B = pa.tile([TH, 512], F32, tag="x1B")
                act_bias(x1A[:], ps1A[:], binA[:])
                act_bias(x1B[:], ps1B[:], binB[:])
                leaky_(x1A[:])
                leaky_(x1B[:])
                emit_node_major(pa, paps, x1A[:], x1B[:], agin1, c)

        # ------------------------------------------------------------
        # RGCN layer: gather + indicator-matmul aggregation + consume
        # ------------------------------------------------------------
        def layer(lid, agin, xg, consume):
            wc = L.wc
            with tc.tile_pool(name=f"gb{lid}", bufs=4) as gb, \
                 tc.tile_pool(name=f"gi{lid}", bufs=4) as gip, \
                 tc.tile_pool(name=f"ind{lid}", bufs=3) as indp, \
                 tc.tile_pool(name=f"agg{lid}", bufs=1) as aggp, \
                 tc.tile_pool(name=f"agps{lid}", bufs=2, space="PSUM") as agps, \
                 tc.tile_pool(name=f"mmps{lid}", bufs=1, space="PSUM") as mmps, \
                 tc.tile_pool(name=f"xt{lid}", bufs=2) as xt, \
                 tc.tile_pool(name=f"cons{lid}", bufs=2) as cpool:
                woff = 0      # global window index
                soff = 0      # global slot offset (for gidx columns)
                for s in range(C.nsup):
                    accg = []
                    for g_ in range(C.ngrp):
                        aA_ = aggp.tile([96, 512], F32, tag=f"accA{g_}",
                                        name="aA_")
                        aB_ = aggp.tile([96, 512], F32, tag=f"accB{g_}",
                                        name="aB_")
                        accg.append((aA_, aB_))
                    for ch in range(C.nchunks):
                        xg_sl = xg[ch * C.chunk_rows:(ch + 1) * C.chunk_rows, :]
                        wlist = []   # (group, first, last) per window
                        for g in range(C.ngrp):
                            n = wc[s][ch][g]
                            for i in range(n):
                                wlist.append((g, i == 0, i == n - 1))
                        # batches of up to 8 windows
                        b = 0
                        psg = {}
                        while b < len(wlist):
                            k = min(8, len(wlist) - b)
                            nb = k * 128
                            gi = gip.tile([P, nb // 16], I16, tag="gi")
                            nc.sync.dma_start(
                                out=gi[:],
                                in_=gidx[:, soff // 16:(soff + nb) // 16])
                            bt = gb.tile([P, k, EMB], F32, tag="bt")
                            nc.gpsimd.dma_gather(bt[:], xg_sl, gi[:],
                                                 nb, nb, EMB)
                            btf = gb.tile([P, k, EMB], F16, tag="btf")
                            nc.scalar.activation(btf[:], bt[:], ACTF.Identity)
                            wsl = slice(woff + b, woff + b + k)
                            nc.vector.tensor_tensor(
                                btf[:], btf[:],
                                inv_sb[:, wsl].unsqueeze(-1).broadcast_to(
                                    [P, k, EMB]),
                                op=AX.mult)
                            ind = indp.tile([P, k, 512], F16, tag="ind")
                            nc.vector.tensor_tensor(
                                ind[:],
                                seg_sb[:, wsl].unsqueeze(-1).broadcast_to(
                                    [P, k, 512]),
                                iota[:, 0:k, :],
                                op=AX.is_equal)
                            for j in range(k):
                                g, first, last = wlist[b + j]
                                if first:
                                    psg[g] = (agps.tile([96, 512], F32,
                                                        tag="psA", name="psA"),
                                              agps.tile([96, 512], F32,
                                                        tag="psB", name="psB"))
                                pA, pB = psg[g]
                                nc.tensor.matmul(pA[:], btf[:, j, 0:96],
                                                 ind[:, j, :],
                                                 start=first, stop=last)
                                nc.tensor.matmul(pB[:], btf[:, j, 96:EMB],
                                                 ind[:, j, :],
                                                 start=first, stop=last)
                                if last:
                                    aA_, aB_ = accg[g]
                                    if ch == 0:
                                        nc.vector.tensor_copy(aA_[:], pA[:])
                                        nc.vector.tensor_copy(aB_[:], pB[:])
                                    else:
                                        nc.vector.tensor_tensor(
                                            aA_[:], aA_[:], pA[:], op=AX.add)
                                        nc.vector.tensor_tensor(
                                            aB_[:], aB_[:], pB[:], op=AX.add)
                                    del psg[g]
                            soff += nb
                            b += k
                        woff += len(wlist)
                    # consume super s: 512-node chunks
                    for i in range(C.sup_d // 512):
                        c = s * (C.sup_d // 512) + i
                        xa, xb = load_xT(xt, mmps, agin, c)
                        oA = mmps.tile([P, 512], F32, tag="oA")
                        oB = mmps.tile([TH, 512], F32, tag="oB")
                        mm(oA[:], wrootA[:, 0:P], xa[:], first=True)
                        mm(oA[:], wrootB[:, 0:P], xb[:])
                        mm(oB[:], wrootA[:, P:EMB], xa[:], first=True)
                        mm(oB[:], wrootB[:, P:EMB], xb[:])
                        for r in range(2):
                            lo, hi = wrel[r]
                            for piece, ai in ((lo, 0), (hi, 1)):
                                last = (r == 1 and ai == 1)
                                csl = slice(r * 256, r * 256 + 256)
                                for h in range(2):
                                    mm(oA[:, bass.ts(h, 256)], piece[:, 0:P],
                                       accg[2 * i + h][ai][:, csl], last=last)
                                for h in range(2):
                                    mm(oB[:, bass.ts(h, 256)], piece[:, P:EMB],
                                       accg[2 * i + h][ai][:, csl], last=last)
                        consume(c, oA, oB, cpool, mmps)

        # ---- layer 1 ----
        nc.gpsimd.collective_compute(
            "AllGather", AX.bypass, replica_groups=replica,
            ins=[agin1[:, :].opt()], outs=[xg1[:, :].opt()])

        def consume1(c, oA, oB, cpool, cpps):
            x2a = cpool.tile([P, 512], F32, tag="x2a")
            x2b = cpool.tile([TH, 512], F32, tag="x2b")
            act_bias(x2a[:], oA[:], brgA[:])
            act_bias(x2b[:], oB[:], brgB[:])
            emit_node_major(cpool, cpps, x2a[:], x2b[:], agin2, c, tagA="oA")

        layer(1, agin1, xg1, consume1)

        # ---- layer 2 (+ fused output head) ----
        nc.gpsimd.collective_compute(
            "AllGather", AX.bypass, replica_groups=replica,
            ins=[agin2[:, :].opt()], outs=[xg2[:, :].opt()])

        def consume2(c, oA, oB, cpool, cpps):
            x3A = cpool.tile([P, 512], F32, tag="x3A")
            x3B = cpool.tile([TH, 512], F32, tag="x3B")
            act_bias(x3A[:], oA[:], brgA[:])
            act_bias(x3B[:], oB[:], brgB[:])
            p1A = cpps.tile([P, 512], F32, tag="oA")
            p1B = cpps.tile([TH, 512], F32, tag="oB")
            mm(p1A[:], wo1A[:, 0:P], x3A[:], first=True)
            mm(p1A[:], wo1B[:, 0:P], x3B[:], last=True)
            mm(p1B[:], wo1A[:, P:EMB], x3A[:], first=True)
            mm(p1B[:], wo1B[:, P:EMB], x3B[:], last=True)
            o1A = cpool.tile([P, 512], F32, tag="o1A")
            o1B = cpool.tile([TH, 512], F32, tag="o1B")
            act_bias(o1A[:], p1A[:], bo1A[:])
            act_bias(o1B[:], p1B[:], bo1B[:])
            leaky_(o1A[:])
            leaky_(o1B[:])
            for t in range(4):
                pso = cpps.tile([P, 2], F32, tag="pso")
                mm(pso[:], o1A[:, bass.ts(t, P)], wo2A[:], first=True)
                mm(pso[:], o1B[:, bass.ts(t, P)], wo2B[:], last=True)
                nc.vector.tensor_tensor(outsb[:, c * 4 + t, :], pso[:],
                                        bo2[:], op=AX.add)

        layer(2, agin2, xg2, consume2)

        nc.sync.dma_start(
            out=out_p[:, :].rearrange("(t p) c -> p t c", p=P),
            in_=outsb[:])

    return nc


# ----------------------------------------------------------------------------
# Host-side structural preprocessing
# ----------------------------------------------------------------------------

def _wrap16(a):
    w = a.reshape(-1, 16).T.astype(np.int16)
    return np.ascontiguousarray(np.tile(w, (8, 1)))


def prep_inputs(C: Cfg, inputs):
    des = np.asarray(inputs["des"], np.float32)
    num_prop = np.asarray(inputs["num_prop"], np.float32)
    cat_prop = np.asarray(inputs["cat_prop"], np.float32)
    ei = np.asarray(inputs["edge_index"]).astype(np.int64)
    et = np.asarray(inputs["edge_type"]).astype(np.int64)
    src, dst = ei[0], ei[1]

    w_rel = np.asarray(inputs["W_rel"], np.float32)
    shared = {
        "w_des": np.asarray(inputs["W_des"], np.float32),
        "w_num": np.asarray(inputs["W_num"], np.float32),
        "w_cat": np.asarray(inputs["W_cat"], np.float32),
        "b0": np.concatenate([np.asarray(inputs["b_des"], np.float32),
                              np.asarray(inputs["b_num"], np.float32),
                              np.asarray(inputs["b_cat"], np.float32)]),
        "w_in": np.asarray(inputs["W_in"], np.float32),
        "b_in": np.asarray(inputs["b_in"], np.float32),
        "w_root": np.asarray(inputs["W_root"], np.float32),
        "w_rel0": np.ascontiguousarray(w_rel[0]),
        "w_rel1": np.ascontiguousarray(w_rel[1]),
        "b_rgcn": np.asarray(inputs["b_rgcn"], np.float32),
        "w_o1": np.asarray(inputs["W_o1"], np.float32),
        "b_o1": np.asarray(inputs["b_o1"], np.float32),
        "w_o2": np.asarray(inputs["W_o2"], np.float32),
        "b_o2r": np.tile(np.asarray(inputs["b_o2"], np.float32)[None, :],
                         (128, 1)),
    }

    n_per = C.n_per
    src_pad = (src // n_per) * C.nsh + (src % n_per)
    chunk_all = (src_pad // C.chunk_rows).astype(np.int64)
    src_loc_all = (src_pad % C.chunk_rows).astype(np.int32)
    dst_core = dst // n_per
    dst_loc_all = (dst % n_per).astype(np.int64)

    NS, NG, NC_ = C.nsup, C.ngrp, C.nchunks
    per_core = []
    cnts = np.zeros((C.ncores, NS, NC_, NG), np.int64)
    for i in range(C.ncores):
        m = dst_core == i
        dl = dst_loc_all[m]
        ch = chunk_all[m]
        sl = src_loc_all[m]
        r = et[m]
        sup = dl // C.sup_d
        grp = (dl % C.sup_d) // C.grp_d
        slot = (r * C.grp_d + (dl % C.grp_d)).astype(np.int64)
        # per (dst, rel) in-degree -> inv
        cnt = np.bincount(dl * 2 + r, minlength=2 * C.nsh)
        inv = (1.0 / np.maximum(cnt, 1)).astype(np.float32)
        inv_e = inv[dl * 2 + r]
        order = np.lexsort((slot, grp, ch, sup))
        key = ((sup * NC_ + ch) * NG + grp)
        np.add.at(cnts[i].reshape(-1), key, 1)
        per_core.append((sl[order], inv_e[order], slot[order],
                         key[order]))

    caps = 128 * np.maximum(
        np.ceil(cnts.max(axis=0) / 128).astype(np.int64), 1)  # [NS,NC_,NG]
    wc = tuple(tuple(tuple(int(x) // 128 for x in c) for c in s)
               for s in caps)
    L = Layout(wc=wc)
    total = int(caps.sum())

    offs = np.zeros(NS * NC_ * NG + 1, np.int64)
    np.cumsum(caps.reshape(-1), out=offs[1:])

    in_maps = []
    for i in range(C.ncores):
        rs, re = i * n_per, (i + 1) * n_per
        desT = np.zeros((C.d_des, C.nsh), np.float32)
        desT[:, :n_per] = des[rs:re].T
        numT = np.zeros((4, C.nsh), np.float32)
        numT[:, :n_per] = num_prop[rs:re].T
        catT = np.zeros((3, C.nsh), np.float32)
        catT[:, :n_per] = cat_prop[rs:re].T

        sl, inv_e, slot, key = per_core[i]
        ne = len(sl)
        # position within run
        kcnt = np.bincount(key, minlength=NS * NC_ * NG)
        kstart = np.zeros(NS * NC_ * NG + 1, np.int64)
        np.cumsum(kcnt, out=kstart[1:])
        pos = np.arange(ne) - kstart[key]
        gpos = offs[key] + pos

        gflat = np.zeros(total, np.int32)
        segf = np.full(total, SEG_PAD, np.float32)
        invf = np.zeros(total, np.float32)
        gflat[gpos] = sl
        segf[gpos] = slot
        invf[gpos] = inv_e

        gidx = _wrap16(gflat)
        segp = np.ascontiguousarray(
            segf.reshape(-1, 128).T.astype(np.float16))  # [128, W]
        invp = np.ascontiguousarray(
            invf.reshape(-1, 128).T.astype(np.float16))

        in_maps.append(dict(shared, desT=desT, numT=numT, catT=catT,
                            gidx=gidx, segp=segp, invp=invp))
    return L, in_maps


_PROG_CACHE = {}


def _get_program(C: Cfg, L: Layout):
    key = (C, L)
    if key not in _PROG_CACHE:
        nc = build_program(C, L)
        nc.finalize()
        _PROG_CACHE[key] = nc
    return _PROG_CACHE[key]


def _install_ntff_shim():
    import types
    try:
        import antenv.axon_hooks  # noqa: F401
        return
    except ImportError:
        pass
    try:
        from trn_agent_boot.trn_boot import _ntff_profile_via_ctypes
        hook = _ntff_profile_via_ctypes("/opt/axon/libaxon_pjrt.so")
    except Exception:
        hook = None
    mod = types.ModuleType("antenv.axon_hooks")
    mod.get_axon_ntff_profile_hook = lambda: hook
    mod.set_axon_ntff_profile_hook = lambda h: None
    sys.modules["antenv.axon_hooks"] = mod


def run_on_hw(inputs, trace=False, trace_kwargs=None):
    C = CFG
    L, in_maps = prep_inputs(C, inputs)
    nc = _get_program(C, L)
    if trace:
        _install_ntff_shim()
    res = run_bass_kernel_spmd(nc, in_maps, list(range(C.ncores)),
                               trace=trace, **(trace_kwargs or {}))
    outs = [res.results[i]["out"][:C.n_per] for i in range(C.ncores)]
    full = np.concatenate(outs, axis=0)
    return full, res


def kernel(**inputs):
    full, _ = run_on_hw(inputs, trace=False)
    return full


# revision 23
# speedup vs baseline: 1.2259x; 1.2090x over previous
"""BotRGCN Trainium2 kernel (8 NeuronCores, SPMD) — v2.

Sharding: nodes row-wise across 8 cores (12800 padded rows/core).
RGCN aggregation: dst-sorted dma_gather (4 source chunks — int16 index
reach) + segment-indicator matmuls on the Tensor engine (PSUM per
dst-group, SBUF f32 accumulator across the 4 chunk passes). No
dma_scatter_add — Q7 descriptor generation (the v1 bottleneck) is
halved. The edge layout (per-group slot capacities) is data-dependent,
so the program is built per input inside kernel(); host preprocessing
remains structural only (sharding, sorting, degree counts).

Self-contained: hardcodes N=100000, E=1600000, EMB=192, 2 relations.
"""

import os
import sys
from contextlib import ExitStack
from dataclasses import dataclass

import numpy as np
import ml_dtypes

for _p in ("/opt/trn_rl_repo",):
    if os.path.isdir(_p) and _p not in sys.path:
        sys.path.insert(0, _p)

import concourse.bass as bass
import concourse.mybir as mybir
from concourse import bacc, library_config, tile
from concourse.bass_utils import run_bass_kernel_spmd

F32 = mybir.dt.float32
BF16 = mybir.dt.bfloat16
F16 = mybir.dt.float16
I16 = mybir.dt.int16
AX = mybir.AluOpType
ACTF = mybir.ActivationFunctionType

LEAKY = 0.01
SEG_PAD = 600  # sentinel outside [0, 512)


@dataclass(frozen=True)
class Cfg:
    ncores: int = 8
    nsh: int = 12800           # padded nodes per core (mult of 2560)
    d_des: int = 768
    emb: int = 192
    third: int = 64
    src_chunk_cores: int = 2   # table rows per chunk <= 32767
    n_real: int = 100000
    sup_d: int = 2560          # dst nodes per super-block
    grp_d: int = 256           # dst nodes per psum group (512 slots)

    @property
    def n_per(self):
        return self.n_real // self.ncores

    @property
    def n_total(self):
        return self.ncores * self.nsh

    @property
    def nchunks(self):
        return self.ncores // self.src_chunk_cores

    @property
    def chunk_rows(self):
        return self.src_chunk_cores * self.nsh

    @property
    def nsup(self):
        return self.nsh // self.sup_d

    @property
    def ngrp(self):
        return self.sup_d // self.grp_d

    @property
    def node_chunks(self):
        return self.nsh // 512

    @property
    def kd(self):
        return self.d_des // 128


CFG = Cfg()


@dataclass(frozen=True)
class Layout:
    """Input-dependent edge layout, identical across cores."""
    wc: tuple            # wc[s][ch][g] -> window count (128 slots each)

    @property
    def total_windows(self):
        return sum(w for s in self.wc for c in s for w in c)

    @property
    def total_slots(self):
        return 128 * self.total_windows


# ----------------------------------------------------------------------------
# Device program
# ----------------------------------------------------------------------------

def build_program(C: Cfg, L: Layout):
    nc = bacc.Bacc(None, num_devices=C.ncores,
                   num_swdge_queues=2, dynamic_dma_scratch_size=32768)

    P = 128
    EMB, TH = C.emb, C.third
    NSH = C.nsh
    NCH = C.node_chunks
    W = L.total_windows

    def param(name, shape, dtype=F32, out=False):
        return nc.declare_dram_parameter(name, list(shape), dtype, isOutput=out)

    desT = param("desT", (C.d_des, NSH))
    numT = param("numT", (4, NSH))
    catT = param("catT", (3, NSH))
    gidx = param("gidx", (P, L.total_slots // 16), I16)
    segp = param("segp", (P, W), F16)
    invp = param("invp", (P, W), F16)
    w_des = param("w_des", (C.d_des, TH))
    w_num = param("w_num", (4, TH))
    w_cat = param("w_cat", (3, TH))
    b0 = param("b0", (EMB,))
    w_in = param("w_in", (EMB, EMB))
    b_in = param("b_in", (EMB,))
    w_root = param("w_root", (EMB, EMB))
    w_rel0 = param("w_rel0", (EMB, EMB))
    w_rel1 = param("w_rel1", (EMB, EMB))
    b_rgcn = param("b_rgcn", (EMB,))
    w_o1 = param("w_o1", (EMB, EMB))
    b_o1 = param("b_o1", (EMB,))
    w_o2 = param("w_o2", (EMB, 2))
    b_o2r = param("b_o2r", (P, 2))
    out_p = param("out", (NSH, 2), out=True)

    agin1 = nc.dram_tensor("agin1", [NSH, EMB], F32)
    xg1 = nc.dram_tensor("xg1", [C.n_total, EMB], F32, addr_space="Shared")
    agin2 = nc.dram_tensor("agin2", [NSH, EMB], F32)
    xg2 = nc.dram_tensor("xg2", [C.n_total, EMB], F32, addr_space="Shared")

    replica = [list(range(C.ncores))]

    with tile.TileContext(nc) as tc, ExitStack() as top:
        nc.gpsimd.load_library(library_config.mlp)
        const = top.enter_context(tc.tile_pool(name="const", bufs=1))

        def cload(src_ap, shape, name, dtype=F32):
            t = const.tile(list(shape), dtype, tag=name)
            nc.sync.dma_start(out=t[:], in_=src_ap)
            return t

        ident_dram = nc.inline_tensor(np.eye(P, dtype=np.float32),
                                      name="identity128")
        ident = cload(ident_dram[:, :], (P, P), "ident")

        def load_ab(w, name):
            a = cload(w[0:P, :], (P, int(w.shape[1])), name + "A")
            b = cload(w[P:EMB, :], (EMB - P, int(w.shape[1])), name + "B")
            return a, b

        def load_lohi(w, name):
            lo = cload(w[0:96, :], (96, int(w.shape[1])), name + "lo")
            hi = cload(w[96:EMB, :], (96, int(w.shape[1])), name + "hi")
            return lo, hi

        wdes = cload(w_des[:, :].rearrange("(k p) m -> p k m", p=P),
                     (P, C.kd, TH), "wdes")
        wnum = cload(w_num[:, :], (4, TH), "wnum")
        wcat = cload(w_cat[:, :], (3, TH), "wcat")
        winA, winB = load_ab(w_in, "win")
        wrootA, wrootB = load_ab(w_root, "wroot")
        wrel = [load_lohi(w_rel0, "wrel0"), load_lohi(w_rel1, "wrel1")]
        wo1A, wo1B = load_ab(w_o1, "wo1")
        wo2A = cload(w_o2[0:P, :], (P, 2), "wo2A")
        wo2B = cload(w_o2[P:EMB, :], (EMB - P, 2), "wo2B")

        def load_colvec(v, name):
            a = const.tile([P, 1], F32, tag=name + "A")
            b = const.tile([EMB - P, 1], F32, tag=name + "B")
            nc.sync.dma_start(out=a[:], in_=v[0:P].unsqueeze(1))
            nc.sync.dma_start(out=b[:], in_=v[P:EMB].unsqueeze(1))
            return a, b

        b0A, b0B = load_colvec(b0, "b0")
        binA, binB = load_colvec(b_in, "bin")
        brgA, brgB = load_colvec(b_rgcn, "brg")
        bo1A, bo1B = load_colvec(b_o1, "bo1")
        bo2 = cload(b_o2r[:, :], (P, 2), "bo2")

        seg_sb = const.tile([P, W], F16, tag="seg")
        nc.sync.dma_start(out=seg_sb[:], in_=segp[:, :])
        inv_sb = const.tile([P, W], F16, tag="inv")
        nc.sync.dma_start(out=inv_sb[:], in_=invp[:, :])

        iota_dram = nc.inline_tensor(
            np.tile(np.arange(512, dtype=np.float16)[None, None, :],
                    (P, 8, 1)),
            name="iota512")
        iota = cload(iota_dram[:, :, :], (P, 8, 512), "iota", dtype=F16)

        outsb = const.tile([P, NSH // P, 2], F32, tag="outsb")

        def leaky_(ap):
            nc.vector.scalar_tensor_tensor(ap, ap, LEAKY, ap,
                                           op0=AX.mult, op1=AX.max)

        def act_bias(out_ap, in_ap, bias_ap):
            nc.scalar.activation(out_ap, in_ap, ACTF.Identity,
                                 bias=bias_ap, scale=1.0)

        def mm(out_ap, l_ap, r_ap, first=False, last=False):
            nc.tensor.matmul(out_ap, l_ap, r_ap, start=first, stop=last)

        def emit_node_major(pool, psum, xa_c, xb_c, dst, c, tagA="ptA"):
            ptA = psum.tile([P, 512], F32, tag=tagA, name="ptA")
            ptB = psum.tile([P, 256], F32, tag="ptB")
            for t in range(4):
                nc.tensor.matmul(ptA[:, bass.ts(t, P)],
                                 xa_c[:, bass.ts(t, P)], ident[:],
                                 is_transpose=True)
                nc.tensor.matmul(ptB[:, bass.ts(t, TH)],
                                 xb_c[:, bass.ts(t, P)], ident[0:TH, 0:TH],
                                 is_transpose=True)
            agsb = pool.tile([P, 4, EMB], F32, tag="agsb")
            nc.vector.tensor_copy(
                agsb[:, :, 0:P], ptA[:].rearrange("p (t f) -> p t f", f=P))
            nc.vector.tensor_copy(
                agsb[:, :, P:EMB], ptB[:].rearrange("p (t f) -> p t f", f=TH))
            nc.sync.dma_start(
                out=dst[c * 512:(c + 1) * 512, :].rearrange(
                    "(t p) f -> p t f", p=P),
                in_=agsb[:])

        def load_xT(pool, psum, src, c):
            nm = pool.tile([P, 4, EMB], F32, tag="nm")
            nc.sync.dma_start(
                out=nm[:],
                in_=src[c * 512:(c + 1) * 512, :].rearrange(
                    "(t p) f -> p t f", p=P))
            pA = psum.tile([P, 512], F32, tag="oA")
            pB = psum.tile([TH, 512], F32, tag="oB")
            for t in range(4):
                nc.tensor.matmul(pA[:, bass.ts(t, P)], nm[:, t, 0:P],
                                 ident[:], is_transpose=True)
                nc.tensor.matmul(pB[:, bass.ts(t, P)], nm[:, t, P:EMB],
                                 ident[:], is_transpose=True)
            xa = pool.tile([P, 512], F32, tag="xTa")
            xb = pool.tile([TH, 512], F32, tag="xTb")
            nc.vector.tensor_copy(xa[:], pA[:])
            nc.vector.tensor_copy(xb[:], pB[:])
            return xa, xb

        # ------------------------------------------------------------
        # Phase A: x1 = leaky(leaky(feats @ W_*) @ W_in + b_in)
        # ------------------------------------------------------------
        with tc.tile_pool(name="pa", bufs=2) as pa, \
             tc.tile_pool(name="paps", bufs=1, space="PSUM") as paps, \
             tc.tile_pool(name="panp", bufs=1) as panp:
            np_sb = panp.tile([4, NSH], F32, tag="np")
            cp_sb = panp.tile([3, NSH], F32, tag="cp")
            nc.sync.dma_start(out=np_sb[:], in_=numT[:, :])
            nc.sync.dma_start(out=cp_sb[:], in_=catT[:, :])
            desT_r = desT[:, :].rearrange("(k p) n -> p k n", p=P)

            for c in range(NCH):
                sl = bass.ts(c, 512)
                des_c = pa.tile([P, C.kd, 512], F32, tag="des")
                nc.sync.dma_start(out=des_c[:], in_=desT_r[:, :, sl])
                psA = paps.tile([P, 512], F32, tag="psA")
                psB = paps.tile([TH, 512], F32, tag="psB")
                for k in range(C.kd):
                    nc.tensor.matmul(psA[0:TH, :], wdes[:, k, :],
                                     des_c[:, k, :],
                                     start=(k == 0), stop=(k == C.kd - 1))
                nc.tensor.matmul(psA[TH:P, :], wnum[:], np_sb[:, sl],
                                 start=True, stop=True)
                nc.tensor.matmul(psB[:], wcat[:], cp_sb[:, sl],
                                 start=True, stop=True)
                x0A = pa.tile([P, 512], F32, tag="x0A")
                x0B = pa.tile([TH, 512], F32, tag="x0B")
                act_bias(x0A[:], psA[:], b0A[:])
                act_bias(x0B[:], psB[:], b0B[:])
                leaky_(x0A[:])
                leaky_(x0B[:])
                ps1A = paps.tile([P, 512], F32, tag="ps1A")
                ps1B = paps.tile([TH, 512], F32, tag="ps1B")
                mm(ps1A[:], winA[:, 0:P], x0A[:], first=True)
                mm(ps1A[:], winB[:, 0:P], x0B[:], last=True)
                mm(ps1B[:], winA[:, P:EMB], x0A[:], first=True)
                mm(ps1B[:], winB[:, P:EMB], x0B[:], last=True)
                x1A = pa.tile([P, 512], F32, tag="x1A")
                x1B = pa.tile([TH, 512], F32, tag="x1B")
                act_bias(x1A[:], ps1A[:], binA[:])
                act_bias(x1B[:], ps1B[:], binB[:])
                leaky_(x1A[:])
                leaky_(x1B[:])
                emit_node_major(pa, paps, x1A[:], x1B[:], agin1, c)

        # ------------------------------------------------------------
        # RGCN layer: gather + indicator-matmul aggregation + consume
        # ------------------------------------------------------------
        def layer(lid, agin, xg, consume):
            wc = L.wc
            with tc.tile_pool(name=f"gb{lid}", bufs=4) as gb, \
                 tc.tile_pool(name=f"gi{lid}", bufs=4) as gip, \
                 tc.tile_pool(name=f"ind{lid}", bufs=3) as indp, \
                 tc.tile_pool(name=f"agg{lid}", bufs=1) as aggp, \
                 tc.tile_pool(name=f"agps{lid}", bufs=2, space="PSUM") as agps, \
                 tc.tile_pool(name=f"mmps{lid}", bufs=1, space="PSUM") as mmps, \
                 tc.tile_pool(name=f"xt{lid}", bufs=2) as xt, \
                 tc.tile_pool(name=f"cons{lid}", bufs=2) as cpool:
                woff = 0      # global window index
                soff = 0      # global slot offset (for gidx columns)
                for s in range(C.nsup):
                    accg = []
                    for g_ in range(C.ngrp):
                        aA_ = aggp.tile([96, 512], F32, tag=f"accA{g_}",
                                        name="aA_")
                        aB_ = aggp.tile([96, 512], F32, tag=f"accB{g_}",
                                        name="aB_")
                        accg.append((aA_, aB_))
                    for ch in range(C.nchunks):
                        xg_sl = xg[ch * C.chunk_rows:(ch + 1) * C.chunk_rows, :]
                        wlist = []   # (group, first, last) per window
                        for g in range(C.ngrp):
                            n = wc[s][ch][g]
                            for i in range(n):
                                wlist.append((g, i == 0, i == n - 1))
                        # batches of up to 8 windows
                        b = 0
                        psg = {}
                        qn = 0
                        while b < len(wlist):
                            k = min(8, len(wlist) - b)
                            nb = k * 128
                            gi = gip.tile([P, nb // 16], I16, tag="gi")
                            nc.sync.dma_start(
                                out=gi[:],
                                in_=gidx[:, soff // 16:(soff + nb) // 16])
                            bt = gb.tile([P, k, EMB], F32, tag="bt")
                            nc.gpsimd.dma_gather(bt[:], xg_sl, gi[:],
                                                 nb, nb, EMB, queue_num=qn)
                            qn = 1 - qn
                            btf = gb.tile([P, k, EMB], F16, tag="btf")
                            nc.scalar.activation(btf[:], bt[:], ACTF.Identity)
                            wsl = slice(woff + b, woff + b + k)
                            nc.vector.tensor_tensor(
                                btf[:], btf[:],
                                inv_sb[:, wsl].unsqueeze(-1).broadcast_to(
                                    [P, k, EMB]),
                                op=AX.mult)
                            ind = indp.tile([P, k, 512], F16, tag="ind")
                            nc.vector.tensor_tensor(
                                ind[:],
                                seg_sb[:, wsl].unsqueeze(-1).broadcast_to(
                                    [P, k, 512]),
                                iota[:, 0:k, :],
                                op=AX.is_equal)
                            for j in range(k):
                                g, first, last = wlist[b + j]
                                if first:
                                    psg[g] = (agps.tile([96, 512], F32,
                                                        tag="psA", name="psA"),
                                              agps.tile([96, 512], F32,
                                                        tag="psB", name="psB"))
                                pA, pB = psg[g]
                                nc.tensor.matmul(pA[:], btf[:, j, 0:96],
                                                 ind[:, j, :],
                                                 start=first, stop=last)
                                nc.tensor.matmul(pB[:], btf[:, j, 96:EMB],
                                                 ind[:, j, :],
                                                 start=first, stop=last)
                                if last:
                                    aA_, aB_ = accg[g]
                                    if ch == 0:
                                        nc.vector.tensor_copy(aA_[:], pA[:])
                                        nc.vector.tensor_copy(aB_[:], pB[:])
                                    else:
                                        nc.vector.tensor_tensor(
                                            aA_[:], aA_[:], pA[:], op=AX.add)
                                        nc.vector.tensor_tensor(
                                            aB_[:], aB_[:], pB[:], op=AX.add)
                                    del psg[g]
                            soff += nb
                            b += k
                        woff += len(wlist)
                    # consume super s: 512-node chunks
                    for i in range(C.sup_d // 512):
                        c = s * (C.sup_d // 512) + i
                        xa, xb = load_xT(xt, mmps, agin, c)
                        oA = mmps.tile([P, 512], F32, tag="oA")
                        oB = mmps.tile([TH, 512], F32, tag="oB")
                        mm(oA[:], wrootA[:, 0:P], xa[:], first=True)
                        mm(oA[:], wrootB[:, 0:P], xb[:])
                        mm(oB[:], wrootA[:, P:EMB], xa[:], first=True)
                        mm(oB[:], wrootB[:, P:EMB], xb[:])
                        for r in range(2):
                            lo, hi = wrel[r]
                            for piece, ai in ((lo, 0), (hi, 1)):
                                last = (r == 1 and ai == 1)
                                csl = slice(r * 256, r * 256 + 256)
                                for h in range(2):
                                    mm(oA[:, bass.ts(h, 256)], piece[:, 0:P],
                                       accg[2 * i + h][ai][:, csl], last=last)
                                for h in range(2):
                                    mm(oB[:, bass.ts(h, 256)], piece[:, P:EMB],
                                       accg[2 * i + h][ai][:, csl], last=last)
                        consume(c, oA, oB, cpool, mmps)

        # ---- layer 1 ----
        nc.gpsimd.collective_compute(
            "AllGather", AX.bypass, replica_groups=replica,
            ins=[agin1[:, :].opt()], outs=[xg1[:, :].opt()])

        def consume1(c, oA, oB, cpool, cpps):
            x2a = cpool.tile([P, 512], F32, tag="x2a")
            x2b = cpool.tile([TH, 512], F32, tag="x2b")
            act_bias(x2a[:], oA[:], brgA[:])
            act_bias(x2b[:], oB[:], brgB[:])
            emit_node_major(cpool, cpps, x2a[:], x2b[:], agin2, c, tagA="oA")

        layer(1, agin1, xg1, consume1)

        # ---- layer 2 (+ fused output head) ----
        nc.gpsimd.collective_compute(
            "AllGather", AX.bypass, replica_groups=replica,
            ins=[agin2[:, :].opt()], outs=[xg2[:, :].opt()])

        def consume2(c, oA, oB, cpool, cpps):
            x3A = cpool.tile([P, 512], F32, tag="x3A")
            x3B = cpool.tile([TH, 512], F32, tag="x3B")
            act_bias(x3A[:], oA[:], brgA[:])
            act_bias(x3B[:], oB[:], brgB[:])
            p1A = cpps.tile([P, 512], F32, tag="oA")
            p1B = cpps.tile([TH, 512], F32, tag="oB")
            mm(p1A[:], wo1A[:, 0:P], x3A[:], first=True)
            mm(p1A[:], wo1B[:, 0:P], x3B[:], last=True)
            mm(p1B[:], wo1A[:, P:EMB], x3A[:], first=True)
            mm(p1B[:], wo1B[:, P:EMB], x3B[:], last=True)
            o1A = cpool.tile([P, 512], F32, tag="o1A")
            o1B = cpool.tile([TH, 512], F32, tag="o1B")
            act_bias(o1A[:], p1A[:], bo1A[:])
            act_bias(o1B[:], p1B[:], bo1B[:])
            leaky_(o1A[:])
            leaky_(o1B[:])
            for t in range(4):
                pso = cpps.tile([P, 2], F32, tag="pso")
                mm(pso[:], o1A[:, bass.ts(t, P)], wo2A[:], first=True)
                mm(pso[:], o1B[:, bass.ts(t, P)], wo2B[:], last=True)
                nc.vector.tensor_tensor(outsb[:, c * 4 + t, :], pso[:],
                                        bo2[:], op=AX.add)

        layer(2, agin2, xg2, consume2)

        nc.sync.dma_start(
            out=out_p[:, :].rearrange("(t p) c -> p t c", p=P),
            in_=outsb[:])

    return nc


# ----------------------------------------------------------------------------
# Host-side structural preprocessing
# ----------------------------------------------------------------------------

def _wrap16(a):
    w = a.reshape(-1, 16).T.astype(np.int16)
    return np.ascontiguousarray(np.tile(w, (8, 1)))


def prep_inputs(C: Cfg, inputs):
    des = np.asarray(inputs["des"], np.float32)
    num_prop = np.asarray(inputs["num_prop"], np.float32)
    cat_prop = np.asarray(inputs["cat_prop"], np.float32)
    ei = np.asarray(inputs["edge_index"]).astype(np.int64)
    et = np.asarray(inputs["edge_type"]).astype(np.int64)
    src, dst = ei[0], ei[1]

    w_rel = np.asarray(inputs["W_rel"], np.float32)
    shared = {
        "w_des": np.asarray(inputs["W_des"], np.float32),
        "w_num": np.asarray(inputs["W_num"], np.float32),
        "w_cat": np.asarray(inputs["W_cat"], np.float32),
        "b0": np.concatenate([np.asarray(inputs["b_des"], np.float32),
                              np.asarray(inputs["b_num"], np.float32),
                              np.asarray(inputs["b_cat"], np.float32)]),
        "w_in": np.asarray(inputs["W_in"], np.float32),
        "b_in": np.asarray(inputs["b_in"], np.float32),
        "w_root": np.asarray(inputs["W_root"], np.float32),
        "w_rel0": np.ascontiguousarray(w_rel[0]),
        "w_rel1": np.ascontiguousarray(w_rel[1]),
        "b_rgcn": np.asarray(inputs["b_rgcn"], np.float32),
        "w_o1": np.asarray(inputs["W_o1"], np.float32),
        "b_o1": np.asarray(inputs["b_o1"], np.float32),
        "w_o2": np.asarray(inputs["W_o2"], np.float32),
        "b_o2r": np.tile(np.asarray(inputs["b_o2"], np.float32)[None, :],
                         (128, 1)),
    }

    n_per = C.n_per
    src_pad = (src // n_per) * C.nsh + (src % n_per)
    chunk_all = (src_pad // C.chunk_rows).astype(np.int64)
    src_loc_all = (src_pad % C.chunk_rows).astype(np.int32)
    dst_core = dst // n_per
    dst_loc_all = (dst % n_per).astype(np.int64)

    NS, NG, NC_ = C.nsup, C.ngrp, C.nchunks
    per_core = []
    cnts = np.zeros((C.ncores, NS, NC_, NG), np.int64)
    for i in range(C.ncores):
        m = dst_core == i
        dl = dst_loc_all[m]
        ch = chunk_all[m]
        sl = src_loc_all[m]
        r = et[m]
        sup = dl // C.sup_d
        grp = (dl % C.sup_d) // C.grp_d
        slot = (r * C.grp_d + (dl % C.grp_d)).astype(np.int64)
        # per (dst, rel) in-degree -> inv
        cnt = np.bincount(dl * 2 + r, minlength=2 * C.nsh)
        inv = (1.0 / np.maximum(cnt, 1)).astype(np.float32)
        inv_e = inv[dl * 2 + r]
        order = np.lexsort((slot, grp, ch, sup))
        key = ((sup * NC_ + ch) * NG + grp)
        np.add.at(cnts[i].reshape(-1), key, 1)
        per_core.append((sl[order], inv_e[order], slot[order],
                         key[order]))

    caps = 128 * np.maximum(
        np.ceil(cnts.max(axis=0) / 128).astype(np.int64), 1)  # [NS,NC_,NG]
    wc = tuple(tuple(tuple(int(x) // 128 for x in c) for c in s)
               for s in caps)
    L = Layout(wc=wc)
    total = int(caps.sum())

    offs = np.zeros(NS * NC_ * NG + 1, np.int64)
    np.cumsum(caps.reshape(-1), out=offs[1:])

    in_maps = []
    for i in range(C.ncores):
        rs, re = i * n_per, (i + 1) * n_per
        desT = np.zeros((C.d_des, C.nsh), np.float32)
        desT[:, :n_per] = des[rs:re].T
        numT = np.zeros((4, C.nsh), np.float32)
        numT[:, :n_per] = num_prop[rs:re].T
        catT = np.zeros((3, C.nsh), np.float32)
        catT[:, :n_per] = cat_prop[rs:re].T

        sl, inv_e, slot, key = per_core[i]
        ne = len(sl)
        # position within run
        kcnt = np.bincount(key, minlength=NS * NC_ * NG)
        kstart = np.zeros(NS * NC_ * NG + 1, np.int64)
        np.cumsum(kcnt, out=kstart[1:])
        pos = np.arange(ne) - kstart[key]
        gpos = offs[key] + pos

        gflat = np.zeros(total, np.int32)
        segf = np.full(total, SEG_PAD, np.float32)
        invf = np.zeros(total, np.float32)
        gflat[gpos] = sl
        segf[gpos] = slot
        invf[gpos] = inv_e

        gidx = _wrap16(gflat)
        segp = np.ascontiguousarray(
            segf.reshape(-1, 128).T.astype(np.float16))  # [128, W]
        invp = np.ascontiguousarray(
            invf.reshape(-1, 128).T.astype(np.float16))

        in_maps.append(dict(shared, desT=desT, numT=numT, catT=catT,
                            gidx=gidx, segp=segp, invp=invp))
    return L, in_maps


_PROG_CACHE = {}


def _get_program(C: Cfg, L: Layout):
    key = (C, L)
    if key not in _PROG_CACHE:
        nc = build_program(C, L)
        nc.finalize()
        _PROG_CACHE[key] = nc
    return _PROG_CACHE[key]


def _install_ntff_shim():
    import types
    try:
        import antenv.axon_hooks  # noqa: F401
        return
    except ImportError:
        pass
    try:
        from trn_agent_boot.trn_boot import _ntff_profile_via_ctypes
        hook = _ntff_profile_via_ctypes("/opt/axon/libaxon_pjrt.so")
    except Exception:
        hook = None
    mod = types.ModuleType("antenv.axon_hooks")
    mod.get_axon_ntff_profile_hook = lambda: hook
    mod.set_axon_ntff_profile_hook = lambda h: None
    sys.modules["antenv.axon_hooks"] = mod


def run_on_hw(inputs, trace=False, trace_kwargs=None):
    C = CFG
    L, in_maps = prep_inputs(C, inputs)
    nc = _get_program(C, L)
    if trace:
        _install_ntff_shim()
    res = run_bass_kernel_spmd(nc, in_maps, list(range(C.ncores)),
                               trace=trace, **(trace_kwargs or {}))
    outs = [res.results[i]["out"][:C.n_per] for i in range(C.ncores)]
    full = np.concatenate(outs, axis=0)
    return full, res


def kernel(**inputs):
    full, _ = run_on_hw(inputs, trace=False)
    return full
